# revision 1
# baseline (speedup 1.0000x reference)
"""Directed message-passing GNN (chemprop-style D-MPNN) on 8 Trainium2 cores.

Strategy (node-range sharding, zero collectives):
  - Host sorts edges by target node and splits nodes into 8 contiguous
    ranges of 12500 (edges follow their target's range, ~E/8 per core).
  - Per core, per 512-edge chunk: indirect-DMA gather x[src] rows,
    PE-transpose them to feature-major, then run all DEPTH=3 iterations of
    the message MLP + GRU entirely on-chip (the recurrence is per-edge
    local).  Matmuls are feature-major ([K,512] fp32, N=512 free dim);
    gate biases ride the free ScalarE activation bias; Wm2 is folded into
    W_ih on the host.
  - Final messages are PE-transposed to edge-major and written to a DRAM
    scratch buffer in target-sorted order.
  - Segment-sum: per 128-node tile, gather the (4-edge-packed) message
    rows covering its edge span, build one-hot matrices on-chip via
    is_equal(seg, iota) and accumulate  msg.T @ onehot  in PSUM, giving
    feature-major node messages directly.  Node MLP + final PE transpose
    complete the output tile.
"""

import sys

sys.path.insert(0, "/opt/trn_rl_repo")

import numpy as np
from contextlib import ExitStack

import concourse.bass as bass
import concourse.mybir as mybir
import concourse.tile as tile
from concourse.bass import IndirectOffsetOnAxis
from concourse.bass_utils import run_bass_kernel_spmd

# ---------------------------------------------------------------- constants
N_NODES = 100000
N_EDGES = 400000
HIDDEN = 128
NODE_FDIM = 133
EDGE_FDIM = 14
DEPTH = 3
NCORES = 8
P = 128
EC = 512                      # edges per message-phase chunk
NPC = N_NODES // NCORES       # 12500 nodes per core
NT = (NPC + P - 1) // P       # 98 node tiles per core
NPAD = NT * P                 # 12544
F32 = mybir.dt.float32
I32 = mybir.dt.int32
AF = mybir.ActivationFunctionType
ALU = mybir.AluOpType


# ------------------------------------------------ walrus sync-wait limit
def _split_multi_waits(nc):
    """This container's walrus encodes at most ONE sync-wait per
    instruction (any ISA struct). Tile attaches several. Split: insert a
    NoOp per extra wait immediately before the instruction on the same
    engine (sequencer stalls on each in turn)."""
    n_split = 0
    for f in nc.m.functions:
        for bb in f.blocks:
            out = []
            for ins in bb.instructions:
                si = getattr(ins, "sync_info", None)
                waits = list(si.on_wait) if si is not None else []
                if len(waits) > 1:
                    for k, w in enumerate(waits[:-1]):
                        out.append(mybir.InstNoOp(
                            name=f"{ins.name}.w{k}",
                            sync_info=mybir.SyncInfo(on_wait=[w], on_update=[]),
                            bass_nofuse=True,
                            engine=ins.engine,
                        ))
                        n_split += 1
                    ins.sync_info = mybir.SyncInfo(
                        on_wait=[waits[-1]], on_update=list(si.on_update)
                    )
                out.append(ins)
            bb.instructions = out
    return n_split


# ------------------------------------------------------------- host prep
def _prep(inputs):
    """Shard / reorder inputs on the host. Returns (in_maps, meta)."""
    x = np.ascontiguousarray(np.asarray(inputs["x"], np.float32))
    ea = np.ascontiguousarray(np.asarray(inputs["edge_attr"], np.float32))
    ei = np.asarray(inputs["edge_index"])
    src = np.asarray(ei[0], np.int64)
    tgt = np.asarray(ei[1], np.int64)

    f64 = np.float64
    Wm1 = np.asarray(inputs["Wm1"], f64)
    bm1 = np.asarray(inputs["bm1"], f64)
    Wm2 = np.asarray(inputs["Wm2"], f64)
    bm2 = np.asarray(inputs["bm2"], f64)
    W_ih = np.asarray(inputs["W_ih"], f64)
    b_ih = np.asarray(inputs["b_ih"], f64)
    W_hh = np.asarray(inputs["W_hh"], f64)
    b_hh = np.asarray(inputs["b_hh"], f64)
    Wn = np.asarray(inputs["Wn"], f64)
    bn = np.asarray(inputs["bn"], f64)
    Wo1 = np.asarray(inputs["Wo1"], f64)
    bo1 = np.asarray(inputs["bo1"], f64)
    Wo2 = np.asarray(inputs["Wo2"], f64)
    bo2 = np.asarray(inputs["bo2"], f64)

    H = HIDDEN
    # Fuse Wm2 into the GRU input projection: gi = h1 @ (Wm2 @ W_ih.T) + (W_ih@bm2 + b_ih)
    W2G = Wm2 @ W_ih.T                     # [128, 384]
    b2g = W_ih @ bm2 + b_ih                # [384]
    bhh_r, bhh_z, bhh_n = b_hh[:H], b_hh[H:2 * H], b_hh[2 * H:]
    b2g_r, b2g_z, b2g_n = b2g[:H], b2g[H:2 * H], b2g[2 * H:]

    # Wm1 row order for the on-chip activation layout:
    #   chunk A = x features 0:128            -> Wm1 rows 14:142
    #   chunk B = [x feats 128:133 | ea | 0]  -> rows 142:147, 0:14, zeros
    #   chunk C = messages                    -> rows 147:275
    WA = Wm1[14:142]
    WB = np.zeros((128, 128), f64)
    WB[0:5] = Wm1[142:147]
    WB[5:19] = Wm1[0:14]
    WC = Wm1[147:275]

    WHH = W_hh.T                            # [128, 384] gate g at cols gH:(g+1)H
    BHN_LHST = np.zeros((128, 128), f64)
    BHN_LHST[0, :] = bhh_n                  # rank-1 bias via ones-row matmul

    WN1 = Wn[0:128]
    WN2 = np.zeros((128, 128), f64)
    WN2[0:5] = Wn[128:133]
    WNM = Wn[133:261]

    def f32c(a):
        return np.ascontiguousarray(np.asarray(a, np.float32))

    def col(v):
        return f32c(np.asarray(v, f64).reshape(128, 1))

    weights = {
        "WA": f32c(WA), "WB": f32c(WB), "WC": f32c(WC),
        "W2G": f32c(W2G), "WHH": f32c(WHH), "BHN_LHST": f32c(BHN_LHST),
        "WN1": f32c(WN1), "WN2": f32c(WN2), "WNM": f32c(WNM),
        "WO1": f32c(Wo1), "WO2": f32c(Wo2),
        "IDN": f32c(np.eye(128)),
        "IOTA": f32c(np.tile(np.arange(128, dtype=f64), (128, 1))),
        "BM1": col(bm1),
        "BR": col(b2g_r + bhh_r),
        "BZP": col(b2g_z + bhh_z),
        "BZN": col(-(b2g_z + bhh_z)),
        "BGN": col(b2g_n),
        "BHN": col(bhh_n),
        "BN": col(bn), "BO1": col(bo1), "BO2": col(bo2),
    }

    # ---- edge sharding by target-node range
    order = np.argsort(tgt, kind="stable")
    tgt_s = tgt[order]
    src_s = src[order]
    bounds = np.searchsorted(tgt_s, NPC * np.arange(NCORES + 1))
    ecounts = np.diff(bounds)
    EPAD = int(np.ceil(ecounts.max() / EC) * EC)
    CH = EPAD // EC
    MROWS = EPAD // 4

    # agg instance count (uniform across cores)
    I = 1
    per_core = []
    for c in range(NCORES):
        lo, hi = bounds[c], bounds[c + 1]
        tl = tgt_s[lo:hi] - NPC * c
        rp = np.searchsorted(tl, P * np.arange(NT + 1))
        r_lo = rp[:-1] // 4
        r_hi = (rp[1:] + 3) // 4
        nrows = np.maximum(r_hi - r_lo, 0)
        inst = np.maximum((nrows + P - 1) // P, 1)
        I = max(I, int(inst.max()))
        per_core.append((lo, hi, tl, rp, r_lo))
    NI = NT * I

    in_maps = []
    for c in range(NCORES):
        lo, hi, tl, rp, r_lo = per_core[c]
        ec = hi - lo
        srcp = np.zeros(EPAD, np.int32)
        srcp[:ec] = src_s[lo:hi]
        srcw = np.ascontiguousarray(srcp.reshape(EPAD // P, P).T)

        eaT = np.zeros((123, EPAD), np.float32)
        eaT[0:14, :ec] = ea[order[lo:hi]].T

        # aggregation gather rows + relative segment ids
        aggidx = np.zeros((P, NI), np.int32)
        aggseg = np.full((P, NI * 4), -1.0, np.float32)
        tlp = np.full(EPAD, 1 << 30, np.int64)
        tlp[:ec] = tl
        for t in range(NT):
            base = r_lo[t]
            for i in range(I):
                k = t * I + i
                rows = base + P * i + np.arange(P)
                valid = rows * 4 < rp[t + 1]
                rows_c = np.where(valid, rows, 0)
                aggidx[:, k] = rows_c
                e = (rows_c[:, None] * 4 + np.arange(4)[None, :])  # [P,4]
                seg = tlp[np.minimum(e, EPAD - 1)] - P * t
                ok = (valid[:, None] & (e >= rp[t]) & (e < rp[t + 1])
                      & (seg >= 0) & (seg < P))
                aggseg[:, 4 * k:4 * k + 4] = np.where(ok, seg, -1).astype(np.float32)

        xT = np.zeros((256, NPAD), np.float32)
        xT[0:133, :NPC] = x[NPC * c:NPC * (c + 1)].T

        m = {
            "x": x,
            "eaT": eaT,
            "srcw": srcw,
            "aggidx": aggidx,
            "aggseg": aggseg,
            "xT": xT,
        }
        m.update(weights)
        in_maps.append(m)

    meta = {"EPAD": EPAD, "CH": CH, "MROWS": MROWS, "I": I, "NI": NI}
    return in_maps, meta


# ------------------------------------------------------------ bass program
def _build(meta):
    EPAD, CH, MROWS, I, NI = (
        meta["EPAD"], meta["CH"], meta["MROWS"], meta["I"], meta["NI"]
    )
    nc = bass.Bass()

    x_e = nc.dram_tensor("x", [N_NODES, NODE_FDIM], F32, kind="ExternalInput")
    eaT_e = nc.dram_tensor("eaT", [123, EPAD], F32, kind="ExternalInput")
    srcw_e = nc.dram_tensor("srcw", [P, EPAD // P], I32, kind="ExternalInput")
    aggidx_e = nc.dram_tensor("aggidx", [P, NI], I32, kind="ExternalInput")
    aggseg_e = nc.dram_tensor("aggseg", [P, NI * 4], F32, kind="ExternalInput")
    xT_e = nc.dram_tensor("xT", [256, NPAD], F32, kind="ExternalInput")
    wnames = ["WA", "WB", "WC", "BHN_LHST", "WN1", "WN2", "WNM", "WO1",
              "WO2", "IDN", "IOTA"]
    w_e = {n: nc.dram_tensor(n, [128, 128], F32, kind="ExternalInput")
           for n in wnames}
    w_e["W2G"] = nc.dram_tensor("W2G", [128, 384], F32, kind="ExternalInput")
    w_e["WHH"] = nc.dram_tensor("WHH", [128, 384], F32, kind="ExternalInput")
    bnames = ["BM1", "BR", "BZP", "BZN", "BGN", "BHN", "BN", "BO1", "BO2"]
    b_e = {n: nc.dram_tensor(n, [128, 1], F32, kind="ExternalInput")
           for n in bnames}
    out_e = nc.dram_tensor("out", [NPAD, HIDDEN], F32, kind="ExternalOutput")
    msg_e = nc.dram_tensor("msg", [EPAD, HIDDEN], F32)  # internal scratch

    # edge-major message buffer viewed as 4-edge-packed rows for gathers
    msg4 = msg_e[:].rearrange("(r k) h -> r (k h)", k=4)
    # chunk-c view matching the transposed SBUF layout [p, j, h]
    msg_w = msg_e[:].rearrange("(c j p) h -> c p j h", j=4, p=P)

    with tile.TileContext(nc) as tc, ExitStack() as es:
        cst = es.enter_context(tc.tile_pool(name="cst", bufs=1))
        W = {}
        for n in wnames:
            W[n] = cst.tile([128, 128], F32, tag=n, name=n)
            nc.sync.dma_start(W[n][:], w_e[n][:])
        for n in ("W2G", "WHH"):
            W[n] = cst.tile([128, 384], F32, tag=n, name=n)
            nc.sync.dma_start(W[n][:], w_e[n][:])
        B = {}
        for n in bnames:
            B[n] = cst.tile([128, 1], F32, tag=n, name=n)
            nc.sync.dma_start(B[n][:], b_e[n][:])
        srcw = cst.tile([P, EPAD // P], I32, tag="srcw")
        nc.sync.dma_start(srcw[:], srcw_e[:])
        aggidx = cst.tile([P, NI], I32, tag="aggidx")
        nc.sync.dma_start(aggidx[:], aggidx_e[:])
        aggseg = cst.tile([P, NI * 4], F32, tag="aggseg")
        nc.sync.dma_start(aggseg[:], aggseg_e[:])
        ones = cst.tile([128, EC], F32, tag="ones")
        nc.vector.memset(ones[:], 0.0)
        nc.vector.memset(ones[0:1, :], 1.0)

        gp = es.enter_context(tc.tile_pool(name="gp", bufs=8))
        ap = es.enter_context(tc.tile_pool(name="ap", bufs=3))
        hp = es.enter_context(tc.tile_pool(name="hp", bufs=2))
        mp = es.enter_context(tc.tile_pool(name="mp", bufs=3))
        np_ = es.enter_context(tc.tile_pool(name="np", bufs=3))
        pp = es.enter_context(tc.tile_pool(name="pp", bufs=8, space="PSUM"))

        def psum(n=EC):
            t = pp.tile([128, 512], F32, tag="bank", name="bank")
            return t[:, :n]

        def mm(out, lhsT, rhs, start, stop):
            nc.tensor.matmul(out, lhsT, rhs, start=start, stop=stop)

        IDN = W["IDN"]

        # ------------------------------------------------ message phase
        for c in range(CH):
            xa = ap.tile([128, EC], F32, tag="xa")
            xb = ap.tile([128, EC], F32, tag="xb")
            nc.sync.dma_start(xb[5:128, :], eaT_e[:, EC * c:EC * (c + 1)])
            psX = psum()
            psY = psum()
            for j in range(4):
                xg = gp.tile([P, NODE_FDIM], F32, tag="xg")
                nc.gpsimd.indirect_dma_start(
                    out=xg[:],
                    out_offset=None,
                    in_=x_e[:],
                    in_offset=IndirectOffsetOnAxis(
                        ap=srcw[:, 4 * c + j:4 * c + j + 1], axis=0
                    ),
                )
                nc.tensor.transpose(
                    psX[:, P * j:P * (j + 1)], xg[:, 0:128], IDN[:]
                )
                nc.tensor.transpose(
                    psY[:5, P * j:P * (j + 1)], xg[:, 128:133], IDN[:]
                )
            nc.scalar.copy(out=xa[:], in_=psX[:])
            nc.vector.tensor_copy(out=xb[0:5, :], in_=psY[:5, :])

            h = None
            for d in range(DEPTH):
                ps_m = psum()
                mm(ps_m, W["WA"][:], xa[:], True, False)
                if d == 0:
                    mm(ps_m, W["WB"][:], xb[:], False, True)
                else:
                    mm(ps_m, W["WB"][:], xb[:], False, False)
                    mm(ps_m, W["WC"][:], h[:], False, True)
                h1 = hp.tile([128, EC], F32, tag="h1")
                nc.vector.tensor_scalar(
                    h1[:], ps_m, B["BM1"][:], 0.0, ALU.add, ALU.max
                )
                ps_gr = psum()
                ps_gz = psum()
                ps_gn = psum()
                if d == 0:
                    mm(ps_gr, W["W2G"][:, 0:128], h1[:], True, True)
                    mm(ps_gz, W["W2G"][:, 128:256], h1[:], True, True)
                    mm(ps_gn, W["W2G"][:, 256:384], h1[:], True, True)
                else:
                    mm(ps_gr, W["W2G"][:, 0:128], h1[:], True, False)
                    mm(ps_gr, W["WHH"][:, 0:128], h[:], False, True)
                    mm(ps_gz, W["W2G"][:, 128:256], h1[:], True, False)
                    mm(ps_gz, W["WHH"][:, 128:256], h[:], False, True)
                    mm(ps_gn, W["W2G"][:, 256:384], h1[:], True, True)
                    ps_hn = psum()
                    mm(ps_hn, W["WHH"][:, 256:384], h[:], True, False)
                    mm(ps_hn, W["BHN_LHST"][:], ones[:], False, True)
                r = hp.tile([128, EC], F32, tag="r")
                nc.scalar.activation(r[:], ps_gr, AF.Sigmoid, bias=B["BR"][:])
                z = hp.tile([128, EC], F32, tag="z")
                tt = hp.tile([128, EC], F32, tag="tt")
                if d == 0:
                    # z-bar = 1 - z ; h' = (1-z) * n  (h == 0)
                    nc.scalar.activation(
                        z[:], ps_gz, AF.Sigmoid, bias=B["BZN"][:], scale=-1.0
                    )
                    nc.vector.tensor_scalar_mul(tt[:], r[:], B["BHN"][:])
                    nc.vector.tensor_tensor(tt[:], tt[:], ps_gn, ALU.add)
                else:
                    nc.scalar.activation(
                        z[:], ps_gz, AF.Sigmoid, bias=B["BZP"][:]
                    )
                    nc.vector.tensor_tensor(tt[:], r[:], ps_hn, ALU.mult)
                    nc.vector.tensor_tensor(tt[:], tt[:], ps_gn, ALU.add)
                n_t = hp.tile([128, EC], F32, tag="n")
                nc.scalar.activation(n_t[:], tt[:], AF.Tanh, bias=B["BGN"][:])
                h_new = hp.tile([128, EC], F32, tag="h")
                if d == 0:
                    nc.vector.tensor_mul(h_new[:], z[:], n_t[:])
                else:
                    nc.vector.tensor_sub(tt[:], h[:], n_t[:])
                    nc.vector.tensor_mul(tt[:], z[:], tt[:])
                    nc.vector.tensor_add(h_new[:], n_t[:], tt[:])
                h = h_new

            psT = psum()
            for j in range(4):
                nc.tensor.transpose(
                    psT[:, P * j:P * (j + 1)], h[:, P * j:P * (j + 1)], IDN[:]
                )
            mout = mp.tile([128, 4, P], F32, tag="mout")
            nc.vector.tensor_copy(
                out=mout[:], in_=psT.rearrange("p (j h) -> p j h", j=4)
            )
            nc.sync.dma_start(msg_w[c], mout[:])

        # -------------------------------------- aggregation + node phase
        for t in range(NT):
            ps_nm = psum(P)
            nmm = 0
            for i in range(I):
                g = gp.tile([P, 512], F32, tag="mg")
                nc.gpsimd.indirect_dma_start(
                    out=g[:],
                    out_offset=None,
                    in_=msg4,
                    in_offset=IndirectOffsetOnAxis(
                        ap=aggidx[:, t * I + i:t * I + i + 1], axis=0
                    ),
                )
                for j in range(4):
                    k = (t * I + i) * 4 + j
                    oh = np_.tile([P, P], F32, tag="oh")
                    nc.vector.tensor_tensor(
                        oh[:],
                        aggseg[:, k:k + 1].to_broadcast([P, P]),
                        W["IOTA"][:],
                        ALU.is_equal,
                    )
                    nmm += 1
                    mm(ps_nm, g[:, P * j:P * (j + 1)], oh[:],
                       nmm == 1, nmm == I * 4)
            nm = np_.tile([P, P], F32, tag="nm")
            nc.vector.tensor_copy(out=nm[:], in_=ps_nm)
            xt1 = np_.tile([P, P], F32, tag="xt1")
            nc.sync.dma_start(xt1[:], xT_e[0:128, P * t:P * (t + 1)])
            xt2 = np_.tile([P, P], F32, tag="xt2")
            nc.sync.dma_start(xt2[:], xT_e[128:256, P * t:P * (t + 1)])
            ps_nr = psum(P)
            mm(ps_nr, W["WN1"][:], xt1[:], True, False)
            mm(ps_nr, W["WN2"][:], xt2[:], False, False)
            mm(ps_nr, W["WNM"][:], nm[:], False, True)
            nr = np_.tile([P, P], F32, tag="nr")
            nc.vector.tensor_scalar_add(nr[:], ps_nr, B["BN"][:])
            ps_o1 = psum(P)
            mm(ps_o1, W["WO1"][:], nr[:], True, True)
            s = np_.tile([P, P], F32, tag="s")
            nc.scalar.activation(s[:], ps_o1, AF.Relu, bias=B["BO1"][:])
            ps_o2 = psum(P)
            mm(ps_o2, W["WO2"][:], s[:], True, True)
            oT = np_.tile([P, P], F32, tag="oT")
            nc.vector.tensor_scalar_add(oT[:], ps_o2, B["BO2"][:])
            ps_of = psum(P)
            nc.tensor.transpose(ps_of, oT[:], IDN[:])
            ob = np_.tile([P, P], F32, tag="ob")
            nc.vector.tensor_copy(out=ob[:], in_=ps_of)
            nc.sync.dma_start(out_e[P * t:P * (t + 1), :], ob[:])

    _split_multi_waits(nc)
    return nc


# ---------------------------------------------------------------- kernel
LAST_RESULT = None  # BassKernelResults of the most recent kernel() call


def kernel(**inputs) -> np.ndarray:
    global LAST_RESULT
    in_maps, meta = _prep(inputs)
    nc = _build(meta)
    res = run_bass_kernel_spmd(nc, in_maps, list(range(NCORES)))
    LAST_RESULT = res
    out = np.concatenate(
        [res.results[c]["out"][:NPC] for c in range(NCORES)], axis=0
    )
    return out.astype(np.float32)


if __name__ == "__main__":
    sys.path.insert(0, "/root/problem")
    import reference

    inputs = {k: np.asarray(v) for k, v in reference.setup_inputs().items()}
    exp = np.asarray(reference.reference(**inputs))
    act = kernel(**inputs)
    err = np.abs(act - exp).max() / (np.abs(exp).max() + 1e-12)
    print("Relative error:", err)



# revision 2
# speedup vs baseline: 1.0689x; 1.0689x over previous
"""Directed message-passing GNN (chemprop-style D-MPNN) on 8 Trainium2 cores.

v2: fp16 everywhere on the PE path (fp32 matmul runs at 1/4 rate), host-side
edge-major staging of x[src]/edge_attr (kills per-chunk indirect gathers +
8 PE transposes + 2 copies), loop engine-balancing of the GRU elementwise
work across ACT/DVE/Pool, PE-injected adds (identity matmuls) instead of DVE
adds, fp16 message scratch, and batched aggregation gathers.

Structure (node-range sharding, zero collectives), as the v1 baseline:
  - Host sorts edges by target node; nodes split into 8 ranges of 12500.
  - Phase 1 (per 512-edge chunk): load pre-staged feature-major fp16 slices
    of [x[src]; edge_attr], run DEPTH=3 message MLP + GRU on-chip, transpose
    to edge-major, write to a fp16 DRAM scratch in target-sorted order.
  - Phase 2 (per 128-node tile): gather 4-edge-packed message rows, one-hot
    segment matmul into PSUM (feature-major node messages), node MLP, output.
"""

import sys

sys.path.insert(0, "/opt/trn_rl_repo")

import numpy as np
from contextlib import ExitStack

import concourse.bass as bass
import concourse.mybir as mybir
import concourse.tile as tile
from concourse.bass import IndirectOffsetOnAxis
from concourse.bass_utils import run_bass_kernel_spmd

# ---------------------------------------------------------------- constants
N_NODES = 100000
N_EDGES = 400000
HIDDEN = 128
NODE_FDIM = 133
EDGE_FDIM = 14
DEPTH = 3
NCORES = 8
P = 128
EC = 512                      # edges per message-phase chunk
NPC = N_NODES // NCORES       # 12500 nodes per core
NT = (NPC + P - 1) // P       # 98 node tiles per core
NPAD = NT * P                 # 12544
XB = EDGE_FDIM + (NODE_FDIM - P)   # 19 rows: [x feats 128:133 | edge_attr]
F32 = mybir.dt.float32
F16 = mybir.dt.float16
I32 = mybir.dt.int32
AF = mybir.ActivationFunctionType
ALU = mybir.AluOpType


# ------------------------------------------------ walrus sync-wait limit
def _split_multi_waits(nc):
    """This container's walrus encodes at most ONE sync-wait per
    instruction. Tile attaches several. Split: insert a NoOp per extra wait
    immediately before the instruction on the same engine."""
    n_split = 0
    for f in nc.m.functions:
        for bb in f.blocks:
            out = []
            for ins in bb.instructions:
                si = getattr(ins, "sync_info", None)
                waits = list(si.on_wait) if si is not None else []
                if len(waits) > 1:
                    for k, w in enumerate(waits[:-1]):
                        out.append(mybir.InstNoOp(
                            name=f"{ins.name}.w{k}",
                            sync_info=mybir.SyncInfo(on_wait=[w], on_update=[]),
                            bass_nofuse=True,
                            engine=ins.engine,
                        ))
                        n_split += 1
                    ins.sync_info = mybir.SyncInfo(
                        on_wait=[waits[-1]], on_update=list(si.on_update)
                    )
                out.append(ins)
            bb.instructions = out
    return n_split


# ------------------------------------------------------------- host prep
def _prep(inputs):
    """Shard / reorder inputs on the host. Returns (in_maps, meta)."""
    x = np.ascontiguousarray(np.asarray(inputs["x"], np.float32))
    ea = np.ascontiguousarray(np.asarray(inputs["edge_attr"], np.float32))
    ei = np.asarray(inputs["edge_index"])
    src = np.asarray(ei[0], np.int64)
    tgt = np.asarray(ei[1], np.int64)

    f64 = np.float64
    Wm1 = np.asarray(inputs["Wm1"], f64)
    bm1 = np.asarray(inputs["bm1"], f64)
    Wm2 = np.asarray(inputs["Wm2"], f64)
    bm2 = np.asarray(inputs["bm2"], f64)
    W_ih = np.asarray(inputs["W_ih"], f64)
    b_ih = np.asarray(inputs["b_ih"], f64)
    W_hh = np.asarray(inputs["W_hh"], f64)
    b_hh = np.asarray(inputs["b_hh"], f64)
    Wn = np.asarray(inputs["Wn"], f64)
    bn = np.asarray(inputs["bn"], f64)
    Wo1 = np.asarray(inputs["Wo1"], f64)
    bo1 = np.asarray(inputs["bo1"], f64)
    Wo2 = np.asarray(inputs["Wo2"], f64)
    bo2 = np.asarray(inputs["bo2"], f64)

    H = HIDDEN
    # Fuse Wm2 into the GRU input projection:
    #   gi = h1 @ (Wm2 @ W_ih.T) + (W_ih @ bm2 + b_ih)
    W2G = Wm2 @ W_ih.T                     # [128, 384]
    b2g = W_ih @ bm2 + b_ih                # [384]
    bhh_r, bhh_z, bhh_n = b_hh[:H], b_hh[H:2 * H], b_hh[2 * H:]
    b2g_r, b2g_z, b2g_n = b2g[:H], b2g[H:2 * H], b2g[2 * H:]

    # Wm1 row order for the staged activation layout:
    #   chunk A = x features 0:128            -> Wm1 rows 14:142
    #   chunk B = [x feats 128:133 | ea]      -> rows 142:147, 0:14
    #   chunk C = messages                    -> rows 147:275
    WA = Wm1[14:142]
    WB = np.concatenate([Wm1[142:147], Wm1[0:14]], axis=0)   # [19, 128]
    WC = Wm1[147:275]
    WHH = W_hh.T                            # [128, 384]

    WN1 = Wn[0:128]
    WN2 = Wn[128:133]                       # [5, 128]
    WNM = Wn[133:261]

    def f16c(a):
        return np.ascontiguousarray(np.asarray(a, np.float16))

    def col(v):
        return np.ascontiguousarray(
            np.asarray(v, f64).reshape(128, 1).astype(np.float32))

    weights = {
        "WA": f16c(WA), "WB": f16c(WB), "WC": f16c(WC),
        "W2G": f16c(W2G), "WHH": f16c(WHH),
        "WN1": f16c(WN1), "WN2": f16c(WN2), "WNM": f16c(WNM),
        "WO1": f16c(Wo1), "WO2": f16c(Wo2),
        "IDN": np.eye(128, dtype=np.float16),
        "IOTA4": np.tile(np.arange(128, dtype=np.float16), (128, 4)),
        "BM1": col(bm1),
        "BR": col(b2g_r + bhh_r),
        "BZP": col(b2g_z + bhh_z),
        "BZN": col(-(b2g_z + bhh_z)),
        "BGN": col(b2g_n),
        "BHN": col(bhh_n),
        "BN": col(bn), "BO1": col(bo1), "BO2": col(bo2),
    }

    # ---- edge sharding by target-node range
    order = np.argsort(tgt, kind="stable")
    tgt_s = tgt[order]
    src_s = src[order]
    bounds = np.searchsorted(tgt_s, NPC * np.arange(NCORES + 1))
    ecounts = np.diff(bounds)
    EPAD = int(np.ceil(ecounts.max() / EC) * EC)
    CH = EPAD // EC

    # --- fixed-slab aggregation with per-tile static bounds: tile t reads
    #     msg4 rows [s0[t], s0[t] + 128*It[t]) where s0/It are the min/max
    #     over cores of that tile's packed-row range (static per build).
    MR4 = EPAD // 4
    rlo_all = np.zeros((NCORES, NT), np.int64)
    rhi_all = np.zeros((NCORES, NT), np.int64)
    per_core = []
    for c in range(NCORES):
        lo, hi = bounds[c], bounds[c + 1]
        tl = tgt_s[lo:hi] - NPC * c
        rp = np.searchsorted(tl, P * np.arange(NT + 1))
        rlo_all[c] = rp[:-1] // 4
        rhi_all[c] = (rp[1:] + 3) // 4
        per_core.append((lo, hi, tl, rp))
    s0 = rlo_all.min(axis=0)
    e1 = rhi_all.max(axis=0)
    It = np.maximum(-(-(e1 - s0) // P), 1)              # instances per tile
    s0 = np.minimum(s0, MR4 - P * It)
    np.clip(s0, 0, None, out=s0)
    koff = np.concatenate([[0], np.cumsum(It)])         # aggseg col offsets
    KTOT = int(koff[-1])

    x16 = x.astype(np.float16)
    ea16 = ea.astype(np.float16)

    in_maps = []
    for c in range(NCORES):
        lo, hi, tl, rp = per_core[c]
        ec = hi - lo

        # staged feature-major edge inputs (host gather, fp16)
        xs = x16[src_s[lo:hi]]                  # [ec, 133]
        xaT = np.zeros((P, EPAD), np.float16)
        xaT[:, :ec] = xs[:, 0:128].T
        xbT = np.zeros((XB, EPAD), np.float16)
        xbT[0:5, :ec] = xs[:, 128:133].T
        xbT[5:XB, :ec] = ea16[order[lo:hi]].T

        # slab-relative segment ids: for tile t, instance i, partition p,
        # packed row = s0[t] + 128*i + p covers edges 4*row .. 4*row+3.
        # aggseg column block (koff[t]+i)*4+j holds edge j's seg on part p.
        tlp = np.full(EPAD, 1 << 30, np.int64)
        tlp[:ec] = tl
        aggseg = np.full((P, KTOT * 4), -1.0, np.float16)
        for t in range(NT):
            it = int(It[t])
            rows = s0[t] + np.arange(P * it)                    # [P*it]
            e = rows[:, None] * 4 + np.arange(4)[None, :]       # [P*it, 4]
            seg = tlp[np.minimum(e, EPAD - 1)] - P * t
            ok = ((e >= rp[t]) & (e < rp[t + 1]) & (seg >= 0) & (seg < P))
            segf = np.where(ok, seg, -1).astype(np.float16)     # [P*it, 4]
            blk = segf.reshape(it, P, 4).transpose(1, 0, 2).reshape(P, it * 4)
            aggseg[:, koff[t] * 4:(koff[t] + it) * 4] = blk

        xt1 = np.zeros((P, NPAD), np.float16)
        xt1[:, :NPC] = x16[NPC * c:NPC * (c + 1), 0:128].T
        xt2 = np.zeros((5, NPAD), np.float16)
        xt2[:, :NPC] = x16[NPC * c:NPC * (c + 1), 128:133].T

        m = {
            "xaT": xaT,
            "xbT": xbT,
            "aggseg": aggseg,
            "xt1": xt1,
            "xt2": xt2,
        }
        m.update(weights)
        in_maps.append(m)

    meta = {"EPAD": EPAD, "CH": CH, "KTOT": KTOT,
            "IT": [int(v) for v in It],
            "KOFF": [int(v) for v in koff],
            "S0": [int(v) for v in s0]}
    return in_maps, meta


# ------------------------------------------------------------ bass program
def _build(meta):
    EPAD, CH, KTOT = meta["EPAD"], meta["CH"], meta["KTOT"]
    S0, IT, KOFF = meta["S0"], meta["IT"], meta["KOFF"]
    IMAX = max(IT)
    nc = bass.Bass()

    xaT_e = nc.dram_tensor("xaT", [P, EPAD], F16, kind="ExternalInput")
    xbT_e = nc.dram_tensor("xbT", [XB, EPAD], F16, kind="ExternalInput")
    aggseg_e = nc.dram_tensor("aggseg", [P, KTOT * 4], F16,
                              kind="ExternalInput")
    xt1_e = nc.dram_tensor("xt1", [P, NPAD], F16, kind="ExternalInput")
    xt2_e = nc.dram_tensor("xt2", [5, NPAD], F16, kind="ExternalInput")

    w_e = {}
    for n in ("WA", "WC", "WN1", "WNM", "WO1", "WO2", "IDN"):
        w_e[n] = nc.dram_tensor(n, [128, 128], F16, kind="ExternalInput")
    w_e["WB"] = nc.dram_tensor("WB", [XB, 128], F16, kind="ExternalInput")
    w_e["WN2"] = nc.dram_tensor("WN2", [5, 128], F16, kind="ExternalInput")
    w_e["W2G"] = nc.dram_tensor("W2G", [128, 384], F16, kind="ExternalInput")
    w_e["WHH"] = nc.dram_tensor("WHH", [128, 384], F16, kind="ExternalInput")
    w_e["IOTA4"] = nc.dram_tensor("IOTA4", [128, 512], F16,
                                  kind="ExternalInput")
    bnames = ["BM1", "BR", "BZP", "BZN", "BGN", "BHN", "BN", "BO1", "BO2"]
    b_e = {n: nc.dram_tensor(n, [128, 1], F32, kind="ExternalInput")
           for n in bnames}
    out_e = nc.dram_tensor("out", [NPAD, HIDDEN], F32, kind="ExternalOutput")
    msg_e = nc.dram_tensor("msg", [EPAD, HIDDEN], F16)  # internal scratch

    # edge-major message buffer viewed as 4-edge-packed rows for gathers
    msg4 = msg_e[:].rearrange("(r k) h -> r (k h)", k=4)
    # chunk-c view matching the transposed SBUF layout [p, j, h]
    msg_w = msg_e[:].rearrange("(c j p) h -> c p j h", j=4, p=P)

    with tile.TileContext(nc) as tc, ExitStack() as es:
        cst = es.enter_context(tc.tile_pool(name="cst", bufs=1))
        W = {}
        for n, e in w_e.items():
            W[n] = cst.tile(list(e.shape), F16, tag=n, name=n)
            nc.sync.dma_start(W[n][:], e[:])
        B = {}
        for n in bnames:
            B[n] = cst.tile([128, 1], F32, tag=n, name=n)
            nc.sync.dma_start(B[n][:], b_e[n][:])
        aggseg = cst.tile([P, KTOT * 4], F16, tag="aggseg")
        nc.sync.dma_start(aggseg[:], aggseg_e[:])

        ap = es.enter_context(tc.tile_pool(name="ap", bufs=6))
        hp = es.enter_context(tc.tile_pool(name="hp", bufs=4))
        mp = es.enter_context(tc.tile_pool(name="mp", bufs=3))
        gp = es.enter_context(tc.tile_pool(name="gp", bufs=2))
        np_ = es.enter_context(tc.tile_pool(name="np", bufs=3))
        # One shared F32 PSUM pool; phase 1 is software-pipelined across
        # chunk pairs so every engine's FIFO stream alternates between two
        # independent chunks (engines can't reorder past a blocked op).
        pp = es.enter_context(tc.tile_pool(name="pp", bufs=7, space="PSUM"))
        pp16 = es.enter_context(
            tc.tile_pool(name="pp16", bufs=1, space="PSUM"))

        def psum(role, n=EC):
            t = pp.tile([128, 512], F32, tag="bank", name="bank")
            return t[:, :n]

        def mm(out, lhsT, rhs, start, stop):
            nc.tensor.matmul(out, lhsT, rhs, start=start, stop=stop)

        IDN = W["IDN"]
        W2G = W["W2G"]
        WHH = W["WHH"]

        # ------------------------------------------------ message phase
        # Chunks are processed in interleaved pairs: each engine's FIFO
        # stream alternates between the two chunks' ops, so a stalled op of
        # one chunk never head-of-line blocks the other's independent work.
        def p1_load(st):
            st["xa"] = ap.tile([P, EC], F16, tag="xa", name="xa")
            nc.sync.dma_start(
                st["xa"][:], xaT_e[:, EC * st["c"]:EC * (st["c"] + 1)])
            st["xb"] = ap.tile([XB, EC], F16, tag="xb", name="xb")
            nc.sync.dma_start(
                st["xb"][:], xbT_e[:, EC * st["c"]:EC * (st["c"] + 1)])

        def p1_mm_m(st, d):
            h = st["h"]
            ps_m = psum("m")
            mm(ps_m, W["WA"][:], st["xa"][:], True, False)
            if d == 0:
                mm(ps_m, W["WB"][:], st["xb"][:], False, True)
            else:
                mm(ps_m, W["WB"][:], st["xb"][:], False, False)
                mm(ps_m, W["WC"][:], h[:], False, True)
                ps_r = psum("r")
                ps_z = psum("z")
                ps_q = psum("q")
                mm(ps_r, WHH[:, 0:128], h[:], True, False)
                mm(ps_z, WHH[:, 128:256], h[:], True, False)
                mm(ps_q, WHH[:, 256:384], h[:], True, True)
                st["ps_r"], st["ps_z"], st["ps_q"] = ps_r, ps_z, ps_q
            st["ps_m"] = ps_m

        def p1_h1(st, d):
            h1 = hp.tile([128, EC], F16, tag="h1", name="h1")
            if d == 0:
                nc.scalar.activation(
                    h1[:], st["ps_m"], AF.Relu, bias=B["BM1"][:])
            else:
                nc.vector.tensor_scalar(
                    h1[:], st["ps_m"], B["BM1"][:], 0.0, ALU.add, ALU.max)
            st["h1"] = h1

        def p1_mm_g(st, d):
            h1 = st["h1"]
            ps_n = psum("n")
            if d == 0:
                ps_r = psum("r")
                ps_z = psum("z")
                mm(ps_r, W2G[:, 0:128], h1[:], True, True)
                mm(ps_z, W2G[:, 128:256], h1[:], True, True)
                mm(ps_n, W2G[:, 256:384], h1[:], True, True)
                st["ps_r"], st["ps_z"] = ps_r, ps_z
            else:
                mm(st["ps_r"], W2G[:, 0:128], h1[:], False, True)
                mm(st["ps_z"], W2G[:, 128:256], h1[:], False, True)
                mm(ps_n, W2G[:, 256:384], h1[:], True, False)
            st["ps_n"] = ps_n

        def p1_sig(st, d):
            r = hp.tile([128, EC], F16, tag="r", name="r")
            nc.scalar.activation(
                r[:], st["ps_r"], AF.Sigmoid, bias=B["BR"][:])
            st["r"] = r
            z = hp.tile([128, EC], F16, tag="z", name="z")
            if d == 0:
                # zb = 1 - z = sigmoid(-(gz + bz))
                nc.scalar.activation(
                    z[:], st["ps_z"], AF.Sigmoid, bias=B["BZN"][:],
                    scale=-1.0)
            else:
                nc.scalar.activation(
                    z[:], st["ps_z"], AF.Sigmoid, bias=B["BZP"][:])
            st["z"] = z

        def p1_u(st, d):
            u = hp.tile([128, EC], F16, tag="u", name="u")
            if d == 0:
                # t0 = r * bhh_n + gi_n   (gh_n = bhh_n at d0)
                nc.vector.scalar_tensor_tensor(
                    u[:], st["r"][:], B["BHN"][:], st["ps_n"],
                    ALU.mult, ALU.add)
            else:
                # u = (gh_n + bhh_n) * r
                nc.vector.scalar_tensor_tensor(
                    u[:], st["ps_q"], B["BHN"][:], st["r"][:],
                    ALU.add, ALU.mult)
                mm(st["ps_n"], IDN[:], u[:], False, True)
            st["u"] = u

        def p1_tanh(st, d):
            n_t = hp.tile([128, EC], F16, tag="n", name="n_t")
            if d == 0:
                nc.scalar.activation(
                    n_t[:], st["u"][:], AF.Tanh, bias=B["BGN"][:])
            else:
                nc.scalar.activation(
                    n_t[:], st["ps_n"], AF.Tanh, bias=B["BGN"][:])
            st["n"] = n_t

        def p1_upd(st, d):
            n_t = st["n"]
            h_new = hp.tile([128, EC], F16, tag="h", name="h_new")
            if d == 0:
                nc.vector.tensor_mul(h_new[:], st["z"][:], n_t[:])
            else:
                dd = hp.tile([128, EC], F16, tag="dd", name="dd")
                nc.vector.tensor_sub(dd[:], st["h"][:], n_t[:])
                ee = hp.tile([128, EC], F16, tag="ee", name="ee")
                nc.vector.tensor_mul(ee[:], st["z"][:], dd[:])
                nc.vector.tensor_add(h_new[:], n_t[:], ee[:])
            st["h"] = h_new

        def p1_out(st):
            ps_t = pp16.tile([128, EC], F16, tag="bank16", name="bank16")
            h = st["h"]
            for j in range(4):
                nc.tensor.transpose(
                    ps_t[:, P * j:P * (j + 1)], h[:, P * j:P * (j + 1)],
                    IDN[:])
            mout = mp.tile([128, 4, P], F16, tag="mout", name="mout")
            psv = ps_t[:].rearrange("p (j h) -> p j h", j=4)
            if st["c"] % 2 == 0:
                nc.scalar.copy(out=mout[:], in_=psv)
            else:
                nc.vector.tensor_copy(out=mout[:], in_=psv)
            nc.sync.dma_start(msg_w[st["c"]], mout[:])

        for cp in range(0, CH, 2):
            grp = [{"c": c, "h": None} for c in range(cp, min(cp + 2, CH))]
            for st in grp:
                p1_load(st)
            for d in range(DEPTH):
                for st in grp:
                    p1_mm_m(st, d)
                for st in grp:
                    p1_h1(st, d)
                for st in grp:
                    p1_mm_g(st, d)
                for st in grp:
                    p1_sig(st, d)
                for st in grp:
                    p1_u(st, d)
                for st in grp:
                    p1_tanh(st, d)
                for st in grp:
                    p1_upd(st, d)
            for st in grp:
                p1_out(st)

        # -------------------------------------- aggregation + node phase
        # tile t reads msg4 rows [S0[t], S0[t] + 128*IT[t]) as a plain DMA
        # slab; host aggseg masks absorb the per-core row offsets. The node
        # MLP runs 4 tiles wide ([128, 512] ops) to amortize op overheads.
        OB = 4                      # tiles per node-MLP/output group
        iota4v = W["IOTA4"][:].rearrange("p (j h) -> p j h", j=4)
        for og in range(0, NT, OB):
            ob_n = min(OB, NT - og)
            nw = ob_n * P
            xt1g = np_.tile([P, nw], F16, tag="xt1g")
            nc.sync.dma_start(xt1g[:], xt1_e[:, P * og:P * og + nw])
            xt2g = np_.tile([5, nw], F16, tag="xt2g")
            nc.sync.dma_start(xt2g[:], xt2_e[:, P * og:P * og + nw])
            ps_nm4 = psum("m", nw)
            for lt in range(ob_n):
                t = og + lt
                it = IT[t]
                mg = gp.tile([P, IMAX, EC], F16, tag="mg")
                slab = msg4[S0[t]:S0[t] + P * it, :].rearrange(
                    "(i p) f -> p i f", p=P)
                nc.sync.dma_start(mg[:, 0:it, :], slab)
                for i in range(it):
                    k = KOFF[t] + i
                    oh = np_.tile([P, 4 * P], F16, tag="oh")
                    nc.vector.tensor_tensor(
                        oh[:].rearrange("p (j h) -> p j h", j=4),
                        aggseg[:, 4 * k:4 * k + 4].to_broadcast([P, 4, P]),
                        iota4v,
                        ALU.is_equal,
                    )
                    for j in range(4):
                        mm(ps_nm4[:, P * lt:P * (lt + 1)],
                           mg[:, i, P * j:P * (j + 1)],
                           oh[:, P * j:P * (j + 1)],
                           i == 0 and j == 0,
                           i == it - 1 and j == 3)
            nm4 = np_.tile([P, nw], F16, tag="nm4")
            nc.vector.tensor_copy(out=nm4[:], in_=ps_nm4)
            ps_nr = psum("r", nw)
            mm(ps_nr, W["WN1"][:], xt1g[:], True, False)
            mm(ps_nr, W["WN2"][:], xt2g[:], False, False)
            mm(ps_nr, W["WNM"][:], nm4[:], False, True)
            nr4 = np_.tile([P, nw], F16, tag="nr4")
            nc.vector.tensor_scalar_add(nr4[:], ps_nr, B["BN"][:])
            ps_o1 = psum("z", nw)
            mm(ps_o1, W["WO1"][:], nr4[:], True, True)
            s4 = np_.tile([P, nw], F16, tag="s4")
            nc.scalar.activation(s4[:], ps_o1, AF.Relu, bias=B["BO1"][:])
            ps_o2 = psum("q", nw)
            mm(ps_o2, W["WO2"][:], s4[:], True, True)
            oT4 = np_.tile([P, nw], F16, tag="oT4")
            nc.scalar.activation(oT4[:], ps_o2, AF.Identity,
                                 bias=B["BO2"][:])
            ps_of = pp16.tile([128, EC], F16, tag="bank16",
                              name="bank16")
            for lt in range(ob_n):
                nc.tensor.transpose(
                    ps_of[:, P * lt:P * (lt + 1)],
                    oT4[:, P * lt:P * (lt + 1)], IDN[:])
            obuf = mp.tile([P, nw], F32, tag="obuf")
            nc.scalar.copy(out=obuf[:], in_=ps_of[:, :nw])
            out_v = out_e[P * og:P * og + nw, :].rearrange(
                "(k p) h -> p k h", p=P)
            nc.sync.dma_start(out_v, obuf[:])

    _split_multi_waits(nc)
    return nc


# ---------------------------------------------------------------- kernel
LAST_RESULT = None  # BassKernelResults of the most recent kernel() call


def kernel(**inputs) -> np.ndarray:
    global LAST_RESULT
    in_maps, meta = _prep(inputs)
    nc = _build(meta)
    res = run_bass_kernel_spmd(nc, in_maps, list(range(NCORES)))
    LAST_RESULT = res
    out = np.concatenate(
        [res.results[c]["out"][:NPC] for c in range(NCORES)], axis=0
    )
    return out.astype(np.float32)


if __name__ == "__main__":
    sys.path.insert(0, "/root/problem")
    import reference

    inputs = {k: np.asarray(v) for k, v in reference.setup_inputs().items()}
    exp = np.asarray(reference.reference(**inputs))
    act = kernel(**inputs)
    err = np.abs(act - exp).max() / (np.abs(exp).max() + 1e-12)
    print("Relative error:", err)
